# revision 11
# baseline (speedup 1.0000x reference)
"""Trainium2 Bass kernel for out = x * exclusive_cumsum(x, axis=time).

Input x: [B=8, T=4096, D=1024] f32. Pure data parallel: batch element b -> core b.

v4 — fp16 I/O, pair-packed blocks, single-matmul scan, ACT-driven carry chain.

Why: the f32 baseline saturated HBM moving 32 MiB/core; fp16 I/O halves that
(rel-err budget 2e-2 vs ~1.2e-3 measured incl. quantization, validated against
a float64 reference in numpy). At the fp16 DMA floor (~50us), per-INSTRUCTION
overhead rules: engine cost ~= free-size cycles + fixed overhead, independent
of partition count (a [1,512] op costs like a [128,512] one, ~0.6us; every
matmul ~0.43us regardless of contraction rows). So the design minimizes
instruction count and keeps the serial carry chain off busy queues.

Layout: time is zero-padded to 4318 = 17 blocks x 254 rows and each block's
rows are PAIR-REVERSED on the host: SBUF tile [128 partitions, 2048] where
partition p holds two consecutive time rows (4KB contiguous DMA lines),
partition order = descending time, partition 0 = the previous block's last
pair (2-row overlap; block 0 gets host-written zero rows). Both x and out use
a per-block HBM layout [17*128, 2048] fp16 (512KB contiguous per block) so
every engine access starts at partition 0; the host strips each block's
partition-0 row and un-flips.

Per block b, per 512-wide chunk j (time scan, one PSUM group of 2 matmuls):
  ps = wte^T @ X_even + wto^T @ X_odd
where wte = strict-lower-triangular + all-ones row 0, wto = strict-lower-
triangular + zero row 0. Row 0 of X_even holds the running carry (the ACT
engine copies ps_prev[0:1] there, f32 PSUM -> fp16 SBUF, its only job), so
  ps[m] = carry + sum_{earlier pairs} (Xe+Xo)   (exclusive pair prefix)
  ps[0] = carry + block total = the NEXT carry  (free, no extra matmul)
Then per chunk on DVE (j=0) / Pool (j=1):
  out_even = Xe * ps;  A = ps + Xe (fp16);  out_odd = Xo * A
and the block stores full-width from DVE/Pool (alternating) so the in-order
ACT queue never couples the carry chain to mul completion.

Budget per core: PE 68 matmuls ~29us, ACT 32 copies ~19us, DVE/Pool ~30us
each, all under the ~49us DMA floor (17.4MB at 358GB/s/core).
"""

import sys

sys.path.insert(0, "/opt/trn_rl_repo")

import numpy as np

B, T, D = 8, 4096, 1024
PAIRS = 127               # data pairs per block (partitions 1..127)
RB = 2 * PAIRS            # 254 time rows per block
NB = 17                   # blocks; RB*NB = 4318 >= T
TP = RB * NB              # padded time
NCH = 2
CH = D // NCH             # 512, one PSUM bank in f32
ROWS = NB * 128           # 2176 rows in the packed device layout

_CACHE = {}


def _flip_index() -> np.ndarray:
    # Block b, flipped row j -> padded time 254b + 2*(126 - j//2) + j%2:
    # pairs reversed within each block, order preserved within a pair.
    j = np.arange(RB)
    base = 2 * (PAIRS - 1 - j // 2) + j % 2
    return (np.arange(NB)[:, None] * RB + base[None, :]).reshape(-1)


_IDXP = _flip_index()


def _weights(np_dtype=np.float16):
    wte = np.tril(np.ones((128, 128), dtype=np_dtype), -1)
    wte[0, :] = 1.0    # row 0 broadcasts the carry held in X_even[0]
    wto = np.tril(np.ones((128, 128), dtype=np_dtype), -1)
    return wte, wto    # wto row 0 stays 0: kills the odd overlap row


def build_nc(num_devices=B):
    """Build the Bass module for one core's packed [2176, 2048] fp16 shard."""
    import concourse.bass as bass
    import concourse.mybir as mybir
    import concourse.tile as tile
    from concourse import bacc

    f32 = mybir.dt.float32
    f16 = mybir.dt.float16

    nc = bacc.Bacc("TRN2", target_bir_lowering=False, debug=False,
                   num_devices=num_devices)
    x = nc.dram_tensor("x", [ROWS, 2 * D], f16, kind="ExternalInput").ap()
    wte = nc.dram_tensor("wte", [128, 128], f16, kind="ExternalInput").ap()
    wto = nc.dram_tensor("wto", [128, 128], f16, kind="ExternalInput").ap()
    out = nc.dram_tensor("out", [ROWS, 2 * D], f16, kind="ExternalOutput").ap()

    with tile.TileContext(nc) as tc:
        with (
            tc.tile_pool(name="wpool", bufs=1) as wpool,
            tc.tile_pool(name="xpool", bufs=8) as xpool,
            tc.tile_pool(name="apool", bufs=2) as apool,
            tc.tile_pool(name="opool", bufs=6) as opool,
            tc.tile_pool(name="pblk", bufs=3,
                         space=bass.MemorySpace.PSUM) as pblk,
        ):
            we = wpool.tile([128, 128], f16, tag="we")
            nc.sync.dma_start(we[:], wte[:])
            wo = wpool.tile([128, 128], f16, tag="wo")
            nc.sync.dma_start(wo[:], wto[:])

            ps_prev = None
            for b in range(NB):
                xt = xpool.tile([128, 2 * D], f16, tag="xt", name=f"xt{b}")
                nc.sync.dma_start(xt[:], x[b * 128:(b + 1) * 128, :])
                if b > 0:
                    for j in range(NCH):
                        # Carry in: fp16 cast of ps_prev row 0 (carry + block
                        # total) into the even overlap row. Tiny per-chunk
                        # DVE copies keyed to per-chunk matmul groups keep
                        # the chain hop short (GPSIMD cannot read PSUM).
                        nc.vector.tensor_copy(
                            xt[0:1, j * CH:(j + 1) * CH],
                            ps_prev[0:1, j * CH:(j + 1) * CH])
                ps = pblk.tile([128, D], f32, tag="ps", name=f"ps{b}")
                for j in range(NCH):
                    jE = slice(j * CH, (j + 1) * CH)           # even chunk j
                    jO = slice(D + j * CH, D + (j + 1) * CH)   # odd chunk j
                    nc.tensor.matmul(ps[:, jE], we[:], xt[:, jE],
                                     start=True, stop=False)
                    nc.tensor.matmul(ps[:, jE], wo[:], xt[:, jO],
                                     start=False, stop=True)
                # Full-width fp16 image of the prefix: ACT's only bulk work;
                # everything downstream runs in 16-bit 2x mode off SBUF.
                p16 = apool.tile([128, D], f16, tag="p16", name=f"p16_{b}")
                nc.scalar.copy(p16[:], ps[:])
                ot = opool.tile([128, 2 * D], f16, tag="ot", name=f"ot{b}")
                a = apool.tile([128, D], f16, tag="a", name=f"a{b}")
                nc.gpsimd.tensor_mul(ot[:, 0:D], xt[:, 0:D], p16[:])
                nc.vector.tensor_add(a[:], p16[:], xt[:, 0:D])
                nc.gpsimd.tensor_mul(ot[:, D:2 * D], a[:], xt[:, D:2 * D])
                # Full-width 512KB store. DMA can only be initiated from
                # gpsimd/SP/ACT; SP would head-of-line-block loads and ACT
                # is kept off the store path so the chain never queues.
                nc.gpsimd.dma_start(out[b * 128:(b + 1) * 128, :], ot[:])
                ps_prev = ps

    nc.compile()
    return nc


def _pack(x16p: np.ndarray) -> np.ndarray:
    """[TP, D] flipped fp16 -> packed [ROWS, 2D] with 2-row overlap."""
    xdev = np.concatenate(
        [np.zeros((2, D), np.float16), x16p], axis=0)      # [TP+2, D]
    blocks = np.empty((NB, 128, 2 * D), np.float16)
    for b in range(NB):
        blocks[b] = xdev[b * RB:b * RB + 256].reshape(128, 2 * D)
    return blocks.reshape(ROWS, 2 * D)


def _in_maps(x: np.ndarray) -> list[dict]:
    wte, wto = _weights()
    x16 = np.asarray(x).astype(np.float16)
    maps = []
    for c in range(B):
        xpad = np.zeros((TP, D), np.float16)
        xpad[:T] = x16[c]
        maps.append({"x": _pack(xpad[_IDXP]), "wte": wte, "wto": wto})
    return maps


def _unpack(o: np.ndarray) -> np.ndarray:
    """Packed [ROWS, 2D] fp16 -> [T, D] f32 (strip row 0, un-flip)."""
    oflip = o.reshape(NB, 128, 2, D)[:, 1:].reshape(TP, D)
    opad = np.empty((TP, D), np.float16)
    opad[_IDXP] = oflip
    return opad[:T].astype(np.float32)


def kernel(x: np.ndarray) -> np.ndarray:
    from concourse.bass_utils import run_bass_kernel_spmd

    x = np.asarray(x)
    assert x.shape == (B, T, D)
    key = "full"
    if key not in _CACHE:
        _CACHE[key] = build_nc()
    nc = _CACHE[key]

    res = run_bass_kernel_spmd(nc, _in_maps(x), core_ids=list(range(B)))
    return np.stack(
        [_unpack(res.results[c]["out"]) for c in range(B)], axis=0)
